# revision 11
# baseline (speedup 1.0000x reference)
"""Octahedral SHT on 8 NeuronCores (Bass/Tile), quarter-folded fp16 design.

Strategy: shard the 192 latitude rings across 8 cores (24 each). The ragged
per-ring DFT is quarter-folded on the host using the cosine/sine symmetries
j<->n-j and j<->n/2-j: the folded x vectors (we/wo/ze/zo, one per m-parity x
re/im quadrant) have n/4+-1 rows, so every ring fits a single K<=101 matmul
chunk and the E matrix shrinks 4x. Tolerance is 2e-2, so all operands are
plain fp16 (measured pipeline error ~4e-4) - no hi/lo splitting.

Phase 1 (per ring): 4 MMs (re/im x even/odd m) -> psum [128 m~, 256 re|im],
rows 0:64 = even m, 64:128 = odd m. Evacuate f32->f16 (ACT/DVE alternate),
bounce through DRAM to transpose ring-index onto partitions.

Phase 2 (per m): out[bev, l] = G'[r, bev].T @ pw[r, l] with exact triangular
l-range (coeffs vanish for l < m). m's are processed in pairs (m~, 127-m~)
so each psum bank holds exactly 2*l(a)+2*l(b) = 258 f32 columns; one strided
evac per 2-bank tile, fp16 triangular output, host sums 8 partials.

Rings are assigned to (core, slot) by sorted length rank: slot s holds rank
8s..8s+7, one per core, so a per-slot staircase row count R_S[s]=101-4s is
SPMD-uniform and cuts the zero-padding DMA ~45%.
"""
import numpy as np

NLAT, LMAX, MMAX = 192, 128, 128
B, V = 2, 64
BF = B * V            # 128 fused batch (b*64+v)
NCORES = 8
SLOTS = 24            # rings per core
JP = 104              # xef partition dim (>= max folded rows 101)
NPTS = 40320
PWCOLS = 129 * 64     # 8256: pair (a, 127-a) always has l_a + l_b = 129
OUTCOLS = 516 * 32    # 16512


def _octa_nlon():
    half = NLAT // 2
    north = np.array([4 * (i + 1) + 16 for i in range(half)], dtype=np.int64)
    return np.concatenate([north, north[::-1]])


def _plan():
    nlon = _octa_nlon()
    order = np.argsort(-nlon, kind="stable")          # ring ids, length desc
    r_s = [int(nlon[order[8 * s]]) // 4 + 1 for s in range(SLOTS)]
    pairs = [(16 * bp + i, 127 - (16 * bp + i))
             for bp in range(4) for i in range(16)]   # m~ pairs, bp-major
    return nlon, order, r_s, pairs


def _true_m(mt):
    return 2 * mt if mt < 64 else 2 * (mt - 64) + 1


def _fold_ring(xr, n):
    """xr [BF, n] f32 -> (we, wo, ze, zo) with q+1, q, q-1, q rows (q=n//4)."""
    h, q = n // 2, n // 4
    u = np.empty((BF, h + 1), np.float32)
    u[:, 0] = xr[:, 0]
    u[:, h] = xr[:, h]
    u[:, 1:h] = xr[:, 1:h] + xr[:, :h:-1]
    v = xr[:, 1:h] - xr[:, :h:-1]                      # j=1..h-1 at col j-1
    jj = np.arange(1, q)
    we = np.empty((BF, q + 1), np.float32)
    we[:, 0] = u[:, 0] + u[:, h]
    we[:, q] = u[:, q]
    we[:, jj] = u[:, jj] + u[:, h - jj]
    wo = np.empty((BF, q), np.float32)
    wo[:, 0] = u[:, 0] - u[:, h]
    wo[:, jj] = u[:, jj] - u[:, h - jj]
    ze = v[:, jj - 1] - v[:, h - jj - 1]               # [BF, q-1]
    zo = np.empty((BF, q), np.float32)
    zo[:, jj - 1] = v[:, jj - 1] + v[:, h - jj - 1]
    zo[:, q - 1] = v[:, q - 1]
    return we, wo, ze, zo


def _build_core_inputs(c, nlon, order, r_s, pairs, offs, x, E_re, E_im, Pw):
    xef = np.zeros((JP, SLOTS, 768), np.float16)
    pw = np.zeros((SLOTS, PWCOLS), np.float16)
    for s in range(SLOTS):
        gid = int(order[8 * s + c])
        n = int(nlon[gid]); q = n // 4; o = int(offs[gid])
        we, wo, ze, zo = _fold_ring(x[:, o:o + n], n)
        xef[0:q + 1, s, 0:128] = we.T
        xef[0:q,     s, 128:256] = wo.T
        xef[0:q - 1, s, 256:384] = ze.T
        xef[0:q,     s, 384:512] = zo.T
        xef[0:q + 1, s, 512:576] = E_re[gid, 0:q + 1, 0::2]
        xef[0:q,     s, 576:640] = E_re[gid, 0:q, 1::2]
        xef[0:q - 1, s, 640:704] = E_im[gid, 1:q, 0::2]
        xef[0:q,     s, 704:768] = E_im[gid, 1:q + 1, 1::2]
        for p, (a, b) in enumerate(pairs):
            ma, mb = _true_m(a), _true_m(b)
            la = 128 - ma
            pw[s, 129 * p:129 * p + la] = Pw[ma:, ma, gid]
            pw[s, 129 * p + la:129 * (p + 1)] = Pw[mb:, mb, gid]
    return {"xef": xef, "pw": pw}


def _build_bass(r_s, pairs):
    import concourse.bass as bass
    import concourse.mybir as mybir
    from concourse import bacc, tile

    dt = mybir.dt
    nc = bacc.Bacc()

    xef_d = nc.dram_tensor("xef", [JP, SLOTS, 768], dt.float16,
                           kind="ExternalInput")
    pw_d = nc.dram_tensor("pw", [SLOTS, PWCOLS], dt.float16,
                          kind="ExternalInput")
    outp_d = nc.dram_tensor("outp", [BF, OUTCOLS], dt.float16,
                            kind="ExternalOutput")
    gdram = nc.dram_tensor("gdram", [SLOTS, 128 * 256], dt.float16)

    with tile.TileContext(nc) as tc:
        with (
            tc.tile_pool(name="xs", bufs=1) as xs_pool,
            tc.tile_pool(name="pws", bufs=1) as pw_pool,
            tc.tile_pool(name="g1", bufs=4) as g1_pool,
            tc.tile_pool(name="gs", bufs=4) as gs_pool,
            tc.tile_pool(name="os", bufs=3) as os_pool,
            tc.tile_pool(name="ps2", bufs=2, space="PSUM") as ps2,
            tc.tile_pool(name="ps1", bufs=4, space="PSUM") as ps1,
        ):
            QS = [nc.sync, nc.gpsimd, nc.scalar]

            xts = []
            for s in range(SLOTS):
                xt = xs_pool.tile([JP, 768], dt.float16, name=f"xt{s}",
                                  tag=f"xt{s}")
                QS[s % 3].dma_start(out=xt[0:r_s[s], :],
                                    in_=xef_d[0:r_s[s], s, :])
                xts.append(xt)
            pw_sb = pw_pool.tile([SLOTS, PWCOLS], dt.float16)
            nc.gpsimd.dma_start(out=pw_sb[:], in_=pw_d[:])

            # ---- phase 1: 24 rings x 4 quadrant MMs ----
            for s in range(SLOTS):
                K = r_s[s]
                xt = xts[s]
                g_ps = ps1.tile([128, 256], dt.float32, tag="g")
                nc.tensor.matmul(g_ps[0:64, 0:128], xt[0:K, 512:576],
                                 xt[0:K, 0:128])
                nc.tensor.matmul(g_ps[64:128, 0:128], xt[0:K, 576:640],
                                 xt[0:K, 128:256])
                nc.tensor.matmul(g_ps[0:64, 128:256], xt[0:K, 640:704],
                                 xt[0:K, 256:384])
                nc.tensor.matmul(g_ps[64:128, 128:256], xt[0:K, 704:768],
                                 xt[0:K, 384:512])
                g_sb = g1_pool.tile([128, 256], dt.float16, tag="ghl")
                if s % 2 == 0:
                    nc.scalar.copy(g_sb[:], g_ps[:])
                else:
                    nc.vector.tensor_copy(g_sb[:], g_ps[:])
                QS[s % 3].dma_start(out=gdram[s], in_=g_sb[:])

            # ---- phase 2: 64 m~ pairs, exact triangular ----
            o_sb = None
            for bp in range(4):
                # split reads by ring-halves: rows 0:12 depend only on the
                # first 12 bounce writes, so they overlap the phase-1 tail
                glo = gs_pool.tile([SLOTS, 4096], dt.float16, tag="glo")
                QS[(2 * bp) % 3].dma_start(
                    out=glo[0:12, :], in_=gdram[0:12, bp * 4096:(bp + 1) * 4096])
                QS[(2 * bp) % 3].dma_start(
                    out=glo[12:24, :], in_=gdram[12:24, bp * 4096:(bp + 1) * 4096])
                ghi = gs_pool.tile([SLOTS, 4096], dt.float16, tag="ghi")
                QS[(2 * bp + 1) % 3].dma_start(
                    out=ghi[0:12, :], in_=gdram[0:12, (7 - bp) * 4096:(8 - bp) * 4096])
                QS[(2 * bp + 1) % 3].dma_start(
                    out=ghi[12:24, :], in_=gdram[12:24, (7 - bp) * 4096:(8 - bp) * 4096])
                for tt in range(8):
                    t = 8 * bp + tt
                    o_ps = ps2.tile([128, 2, 512], dt.float32, tag="o")
                    for b2 in range(2):
                        p = 2 * t + b2
                        a, _ = pairs[p]
                        i2 = a - 16 * bp
                        la = 128 - 2 * a
                        lb = 129 - la
                        po = 129 * p
                        nc.tensor.matmul(
                            o_ps[:, b2, 0:la],
                            glo[:, i2 * 256:i2 * 256 + 128],
                            pw_sb[:, po:po + la])
                        nc.tensor.matmul(
                            o_ps[:, b2, la:2 * la],
                            glo[:, i2 * 256 + 128:i2 * 256 + 256],
                            pw_sb[:, po:po + la])
                        nc.tensor.matmul(
                            o_ps[:, b2, 2 * la:2 * la + lb],
                            ghi[:, (15 - i2) * 256:(15 - i2) * 256 + 128],
                            pw_sb[:, po + la:po + 129])
                        nc.tensor.matmul(
                            o_ps[:, b2, 2 * la + lb:258],
                            ghi[:, (15 - i2) * 256 + 128:(15 - i2) * 256 + 256],
                            pw_sb[:, po + la:po + 129])
                    if t % 2 == 0:
                        o_sb = os_pool.tile([128, 1032], dt.float16, tag="ot")
                    dst = o_sb[:, (t % 2) * 516:(t % 2 + 1) * 516]
                    if t % 2 == 0:
                        nc.scalar.copy(dst, o_ps[:, :, 0:258])
                    else:
                        nc.vector.tensor_copy(dst, o_ps[:, :, 0:258])
                    if t % 2 == 1:
                        QS[(t // 2) % 3].dma_start(
                            out=outp_d[:, (t - 1) * 516:(t + 1) * 516],
                            in_=o_sb[:])

    nc.compile()
    return nc


_CACHE = {}


def _get_compiled(r_s, pairs):
    if "nc" not in _CACHE:
        _CACHE["nc"] = _build_bass(r_s, pairs)
    return _CACHE["nc"]


def kernel(data, Pw, E_re, E_im, pad_idx):
    from concourse import bass_utils

    data = np.asarray(data)
    Pw = np.asarray(Pw, dtype=np.float32)
    E_re = np.asarray(E_re, dtype=np.float32)
    E_im = np.asarray(E_im, dtype=np.float32)

    nlon, order, r_s, pairs = _plan()
    offs = np.concatenate([[0], np.cumsum(nlon)[:-1]])
    # 'b e p v -> (b e v) p'
    x = np.ascontiguousarray(
        np.transpose(data, (0, 1, 3, 2)).reshape(BF, NPTS).astype(np.float32))

    in_maps = [
        _build_core_inputs(c, nlon, order, r_s, pairs, offs, x, E_re, E_im, Pw)
        for c in range(NCORES)
    ]

    nc = _get_compiled(r_s, pairs)
    res = bass_utils.run_bass_kernel_spmd(nc, in_maps, list(range(NCORES)))
    _CACHE["last_results"] = res

    total = np.zeros((BF, OUTCOLS), np.float64)
    for r in res.results:
        total += r["outp"].astype(np.float64)

    coeffs = np.zeros((LMAX, MMAX, BF), np.complex128)
    for t in range(32):
        for b2 in range(2):
            p = 2 * t + b2
            a, b = pairs[p]
            ma, mb = _true_m(a), _true_m(b)
            la, lb = 128 - ma, 128 - mb
            base = 516 * t + 258 * b2
            re_a = total[:, base:base + la]
            im_a = total[:, base + la:base + 2 * la]
            re_b = total[:, base + 2 * la:base + 2 * la + lb]
            im_b = total[:, base + 2 * la + lb:base + 258]
            coeffs[ma:, ma, :] = (re_a + 1j * im_a).T
            coeffs[mb:, mb, :] = (re_b + 1j * im_b).T
    cc = coeffs.reshape(LMAX, MMAX, B, V)
    out = np.transpose(cc, (2, 0, 1, 3))[:, None]    # [b, 1, l, m, v]
    return out.astype(np.complex64)
